# revision 63
# baseline (speedup 1.0000x reference)
"""Multi-head attention (B=8, N=1024, C=768, H=12) on 8 TRN2 NeuronCores.

Sharding: data-parallel - one batch element per core, weights replicated.
No collectives.

Design (~187us HW, vs 589us f32r baseline): bf16 matmul operands everywhere
(f32 PSUM - TRN2 matmul output must be fp32, so every matmul is N<=512 /
one PSUM bank), query dim split in 512-halves so PSUM fits exactly 8 banks
(st 2x[128,1024] + ov 2x[65,512] + 2 accumulator slots) with QKV interleave
slots, software-pipelined S->exp->PV per j-tile, QKV/V generation for the
next pair interleaved into the attention loop as slack-sized PE gap-filler
(keeps HAM warm and the exp stream dense), reciprocal via the fast
custom-DVE approx (DVE reciprocal is 8 cyc/elem ~ 6.5us per row), partition
broadcast on GPSIMD instead of 7 chained DMAs, input DMAs on the Sync queue
split two-ways per tensor for multi-queue fan-out.

Per-core dataflow:
  qt/kt [128, pair, 1024]: rows = head-pair dims (A at 0:64, B at 64:128).
  v_sb [128 j, jt, head, 65]: col 64 is ones -> PV row 64 = softmax denom.
  Per (pair t, half ib, jtile): S^T halves via two K=64 matmuls row-packed
  into the PE halves, one exp ACTIVATE (FD=1024, scale fused), PV accumulates
  O^T[65, 512] over jt. Normalize: denom row -> reciprocal_approx_fast ->
  partition_broadcast -> multiply into ot_sb (bf16).
  Proj: y = OT.T @ wp + bias per 128-row tile, straight to DRAM.
"""

from contextlib import ExitStack

import numpy as np

import concourse.bacc as bacc
import concourse.mybir as mybir
import concourse.tile as tile
from concourse.bass_utils import run_bass_kernel_spmd

F32 = mybir.dt.float32
BF16 = mybir.dt.bfloat16

B, N, C = 8, 1024, 768
H, HD = 12, 64
SCALE = HD ** -0.5
NT_I = N // 128   # 8 i/j tiles
NT_C = C // 128   # 6 c tiles (== head pairs)
NPAIR = H // 2    # 6


def build():
    nc = bacc.Bacc(None, target_bir_lowering=False)

    xt = nc.dram_tensor("xt", [C, N], BF16, kind="ExternalInput")
    wq = nc.dram_tensor("wq", [C, C], BF16, kind="ExternalInput")
    wk = nc.dram_tensor("wk", [C, C], BF16, kind="ExternalInput")
    wv = nc.dram_tensor("wv", [C, C], BF16, kind="ExternalInput")
    wp = nc.dram_tensor("wp", [C, C], BF16, kind="ExternalInput")
    bias = nc.dram_tensor("bias", [128, C], F32, kind="ExternalInput")
    y = nc.dram_tensor("y", [N, C], F32, kind="ExternalOutput")

    with tile.TileContext(nc) as tc, ExitStack() as stack:
        pp = stack.enter_context(tc.tile_pool(name="persist", bufs=1))
        p_pt = stack.enter_context(tc.tile_pool(name="pt", bufs=16))
        p_nrm = stack.enter_context(tc.tile_pool(name="nrm", bufs=4))
        p_y = stack.enter_context(tc.tile_pool(name="yout", bufs=8))
        ps_qkv = stack.enter_context(
            tc.tile_pool(name="psq", bufs=2, space="PSUM"))
        ps_st = stack.enter_context(
            tc.tile_pool(name="psst", bufs=1, space="PSUM"))
        ps_ov = stack.enter_context(
            tc.tile_pool(name="psov", bufs=1, space="PSUM"))

        xt_sb = pp.tile([128, NT_C, N], BF16)
        wq_sb = pp.tile([128, NT_C, C], BF16)
        wk_sb = pp.tile([128, NT_C, C], BF16)
        wv_sb = pp.tile([128, NT_C, C], BF16)
        wp_sb = pp.tile([128, NT_C, C], BF16)
        bias_sb = pp.tile([128, C], F32)
        qt_sb = pp.tile([128, NPAIR, N], BF16)
        kt_sb = pp.tile([128, NPAIR, N], BF16)
        v_sb = pp.tile([128, NT_I, H, HD + 1], BF16)
        ot_sb = pp.tile([128, NPAIR, N], BF16)

        # Input DMAs: all on the Sync queue (issue on Scalar/GpSimd
        # injects DMA sem-waits into queues carrying exp /
        # partition_broadcast and measured slower). Each tensor is split
        # in two so the transfers fan out across more HW DGE queues
        # (aggregate ~330GB/s vs ~220 for monolithic), in consumption
        # order.
        def dma_k2(dst, src):
            """Split along the k (row-chunk) axis - two HW queues."""
            h = NT_C // 2
            nc.sync.dma_start(
                dst[:, 0:h], src[0:h * 128].rearrange("(t p) i -> p t i",
                                                      p=128))
            nc.sync.dma_start(
                dst[:, h:NT_C],
                src[h * 128:NT_C * 128].rearrange("(t p) i -> p t i", p=128))

        # (column-sliced weight DMAs were tried here: they corrupt on HW
        # - strided multi-queue transfer vs completion-sem race - and are
        # slow; keep contiguous k-splits)
        dma_k2(xt_sb, xt)
        dma_k2(wq_sb, wq)
        dma_k2(wk_sb, wk)
        dma_k2(wv_sb, wv)
        dma_k2(wp_sb, wp)
        nc.sync.dma_start(bias_sb[:], bias[:])

        # HAM warmup: the input-DMA window leaves the PE silent for >3.4us,
        # so the clock gate drops to K=4/8 and (run-dependent) the whole
        # matmul stream can stay ~20% slow (measured 186 vs 218us runs).
        # A burst of dummy matmuls on zeroed tiles keeps the array busy
        # from the end of the engine preamble until the chunk-paced QKV
        # chains take over (~17us).
        warm_a = pp.tile([128, 128], BF16)
        warm_b = pp.tile([128, 512], BF16)
        nc.vector.memset(warm_a[:], 0.0)
        nc.vector.memset(warm_b[:], 0.0)
        for w in range(28):
            warm_ps = ps_st.tile([128, 512], F32, tag=f"st{w % 2}",
                                 name=f"warm{w}")
            nc.tensor.matmul(warm_ps[:], warm_a[:], warm_b[:])
        nc.vector.memset(v_sb[:, :, :, HD:HD + 1], 1.0)

        def gen_qk_chunk(t, which, ch, ks=None, acc=None):
            """One accumulation chain of Q.T (which=0) or K.T (which=1).
            With ks given, emits only those k-steps of the chain (the
            caller threads the acc tile through) - used to granularize
            filler work to the per-step PE slack."""
            w_sb, out_sb = ((wq_sb, qt_sb), (wk_sb, kt_sb))[which]
            if acc is None:
                acc = ps_qkv.tile([128, 512], F32, tag="acc",
                                  name=f"qk{t}_{which}_{ch}")
            for k in (ks if ks is not None else range(NT_C)):
                nc.tensor.matmul(
                    acc[:],
                    w_sb[:, k, t * 128:(t + 1) * 128],
                    xt_sb[:, k, ch * 512:(ch + 1) * 512],
                    start=(k == 0), stop=(k == NT_C - 1),
                )
            if ks is None or ks[-1] == NT_C - 1:
                nc.vector.tensor_copy(out_sb[:, t, ch * 512:(ch + 1) * 512],
                                      acc[:])
            return acc

        def qk_halves(t, which, ch):
            """Two filler pieces continuing one accumulation chain."""
            state = {}

            def first():
                state["acc"] = gen_qk_chunk(t, which, ch, ks=[0, 1, 2])

            def second():
                gen_qk_chunk(t, which, ch, ks=[3, 4, 5], acc=state["acc"])

            return [first, second]

        def gen_v_chunk(jt, ch):
            acc = ps_qkv.tile([128, 384], F32, tag="acc",
                              name=f"v{jt}_{ch}")
            for k in range(NT_C):
                nc.tensor.matmul(
                    acc[:],
                    xt_sb[:, k, jt * 128:(jt + 1) * 128],
                    wv_sb[:, k, ch * 384:(ch + 1) * 384],
                    start=(k == 0), stop=(k == NT_C - 1),
                )
            nc.vector.tensor_copy(
                v_sb[:, jt, 6 * ch:6 * ch + 6, 0:HD],
                acc[:].rearrange("p (h e) -> p h e", e=HD),
            )

        def attn_pair(t, ib, filler, defer=False, mid=None):
            """Attention for head pair t on query half ib (512 queries).

            filler: list of zero-arg emitters (extra PE work) drained a few
            per jt step so the scheduler has gap-fill matmuls while ACT
            runs exp. Emitted at the BOTTOM of each step so they rank below
            the attention instructions in scheduler priority (pure
            gap-fill). Drained fast enough that all run by step NT_I-2.
            """
            i0 = ib * 512
            hA, hB = 2 * t, 2 * t + 1
            per_step = -(-len(filler) // NT_I) if filler else 0
            # separate tags (1 slot each) pin A->A / B->B slot reuse
            # across blocks: the next block's PV_A waits only on the
            # earlier-freed A copies, never on B's
            ovA = ps_ov.tile([HD + 1, 512], F32, tag="ovA",
                             name=f"ovA{t}_{ib}")
            ovB = ps_ov.tile([HD + 1, 512], F32, tag="ovB",
                             name=f"ovB{t}_{ib}")
            pts = [None] * NT_I
            # defer=True (pair 0): all S+exp emitted before any PV, so
            # DMA-gated V tiles / PVs sit behind the exp-feeding matmuls
            # in the in-order PE queue instead of head-of-line blocking
            # them while inputs stream in.
            pv_at = NT_I if defer else 1
            for jt in range(NT_I + pv_at):
                if jt < NT_I:
                    if mid is not None and jt == 4:
                        # work needed from jt=4 on (e.g. pair 0's ch1
                        # Q.T/K.T chains, gated on the late second-half
                        # weight DMAs) is emitted here so it cannot
                        # FIFO-block the first four S steps
                        for f in mid:
                            f()
                    # alternating tags (1 buf each) force round-robin slot
                    # reuse: jt's matmuls wait exp(jt-2), never exp(jt-1)
                    st = ps_st.tile([128, 1024], F32, tag=f"st{jt % 2}",
                                    name=f"st{t}_{ib}_{jt}")
                    nc.tensor.matmul(
                        st[:, 0:512],
                        kt_sb[0:64, t, jt * 128:(jt + 1) * 128],
                        qt_sb[0:64, t, i0:i0 + 512],
                    )
                    nc.tensor.matmul(
                        st[:, 512:1024],
                        kt_sb[64:128, t, jt * 128:(jt + 1) * 128],
                        qt_sb[64:128, t, i0:i0 + 512],
                    )
                    pt = p_pt.tile([128, 1024], BF16, tag="pt")
                    nc.scalar.activation(
                        pt[:], st[:],
                        mybir.ActivationFunctionType.Exp, scale=SCALE,
                    )
                    pts[jt] = pt
                if jt >= pv_at:
                    j = jt - pv_at
                    if defer:
                        # fillers (V tiles for pair 0) drain at the TOP of
                        # the PV phase so v[j] is emitted before its PV and
                        # DMA-gated chains sit behind the whole exp stream
                        for _ in range(per_step):
                            if filler:
                                filler.pop(0)()
                    pt = pts[j]
                    nc.tensor.matmul(
                        ovA[:], v_sb[:, j, hA, :], pt[:, 0:512],
                        start=(j == 0), stop=(j == NT_I - 1),
                    )
                    nc.tensor.matmul(
                        ovB[:], v_sb[:, j, hB, :], pt[:, 512:1024],
                        start=(j == 0), stop=(j == NT_I - 1),
                    )
                if not defer:
                    for _ in range(per_step):
                        if filler:
                            filler.pop(0)()
            while filler:
                filler.pop(0)()
            # normalize: copy the unnormalized O^T + denom row out first so
            # the ov PSUM banks free ~1.3us after the last PV instead of
            # after the whole recip->broadcast->mul chain (~3us) - that
            # chain stalled the next (t, ib)'s PV/exp refill by ~1.6us per
            # boundary. The multiply then runs in-place in SBUF whenever
            # DVE has slack.
            for base, ov in ((0, ovA), (64, ovB)):
                osl = ot_sb[base:base + 64, t, i0:i0 + 512]
                rl = p_nrm.tile([1, 512], F32, tag="rl")
                rc = p_nrm.tile([1, 512], F32, tag="rc")
                bc = p_nrm.tile([128, 512], F32, tag="bc")
                nc.vector.tensor_copy(rl[0:1, :], ov[64:65, :])
                nc.vector.tensor_copy(osl, ov[0:64, :])
                nc.vector.reciprocal_approx_fast(rc[0:1, :], rl[0:1, :])
                nc.gpsimd.partition_broadcast(bc[:], rc[0:1, :])
                nc.vector.tensor_mul(osl, osl, bc[base:base + 64, :])

        y_part = pp.tile([128, 4, C], F32)

        def proj(it, ks, first=True, last=True):
            """Projection of row-tile it over c_in chunks ks. Row-tiles
            4-7 run after the last exp, so their accumulators use the
            then-free st banks - 4 chains in flight instead of 2, keeping
            the PE streaming while the DVE adds drain."""
            y_sb = (p_y.tile([128, C], F32, tag="y", name=f"y{it}")
                    if last else None)
            for ch in range(2):
                if it >= 4:
                    # st AND ov banks are all free after the last exp/norm:
                    # spread the 8 tail chains across 4 slots
                    pool, tag = ((ps_st, "st0"), (ps_st, "st1"),
                                 (ps_ov, "ovA"), (ps_ov, "ovB"))[
                                     (it % 2) * 2 + ch]
                    acc = pool.tile([128, 384], F32, tag=tag,
                                    name=f"p{it}_{ks[0]}_{ch}")
                else:
                    acc = ps_qkv.tile([128, 384], F32, tag="acc",
                                      name=f"p{it}_{ks[0]}_{ch}")
                for k in ks:
                    nc.tensor.matmul(
                        acc[:],
                        ot_sb[:, k, it * 128:(it + 1) * 128],
                        wp_sb[:, k, ch * 384:(ch + 1) * 384],
                        start=(k == ks[0]), stop=(k == ks[-1]),
                    )
                sl = slice(ch * 384, (ch + 1) * 384)
                prev = bias_sb if first else y_part[:, it - 4, :]
                dst = y_sb if last else y_part[:, it - 4, :]
                nc.vector.tensor_add(dst[:, sl], acc[:], prev[:, sl])
            if last:
                eng = (nc.sync, nc.gpsimd)[it % 2]
            eng.dma_start(y[it * 128:(it + 1) * 128, :], y_sb[:])

        # prologue: only the first-half (ch0) Q.T/K.T chains for pair 0;
        # S steps jt<4 need just these. The ch1 chains ride mid-loop (V
        # rides in the fillers) so late DMAs cannot block the first S
        # matmuls in PE order.
        gen_qk_chunk(0, 0, 0)
        gen_qk_chunk(0, 1, 0)
        mid0 = [lambda w=w: gen_qk_chunk(0, w, 1) for w in range(2)]

        for t in range(NPAIR):
            # fillers: extra PE chains emitted at the bottom of the
            # attention jt steps so the scheduler has gap-fill matmuls
            # while ACT runs exp.
            f0, f1 = [], []
            if t == 0:
                # V tiles inside pair 0; V[j] lands before the PV reads it
                f0 = [lambda jt=jt, ch=ch: gen_v_chunk(jt, ch)
                      for jt in range(NT_I) for ch in range(2)]
            if t + 1 < NPAIR:
                for ch in range(2):
                    f0 += qk_halves(t + 1, 0, ch)
                    f1 += qk_halves(t + 1, 1, ch)
            attn_pair(t, 0, f0, defer=(t == 0),
                      mid=(mid0 if t == 0 else None))
            attn_pair(t, 1, f1, defer=(t == 0))

        # projection tail. No point distributing it into the attention
        # loops: PE is ~96% busy there, and any ready filler outranks
        # later attention steps in the list scheduler, starving ACT.
        for it in range(NT_I):
            proj(it, list(range(NT_C)))

    nc.compile()
    nc.finalize()
    return nc


_NC_CACHE = {}


def _get_nc(mode=None):
    if "nc" not in _NC_CACHE:
        _NC_CACHE["nc"] = build()
    return _NC_CACHE["nc"]


def _prep_host(x, w_qkv, w_proj, b_proj, mode=None):
    import ml_dtypes
    bf16 = ml_dtypes.bfloat16

    x = np.asarray(x)
    w_qkv = np.asarray(w_qkv)
    w_proj = np.asarray(w_proj)
    b_proj = np.asarray(b_proj)
    xt = np.ascontiguousarray(x.transpose(0, 2, 1)).astype(bf16)  # [B, C, N]
    wq_t = np.ascontiguousarray(w_qkv[0:C].T).astype(bf16)   # [C, C]
    wk_t = np.ascontiguousarray(w_qkv[C:2 * C].T).astype(bf16)
    wv_t = np.ascontiguousarray(w_qkv[2 * C:3 * C].T).astype(bf16)
    wp_t = np.ascontiguousarray(w_proj.T).astype(bf16)
    bias_rep = np.ascontiguousarray(
        np.broadcast_to(np.asarray(b_proj, dtype=np.float32), (128, C)))
    return xt, wq_t, wk_t, wv_t, wp_t, bias_rep


def run(x, w_qkv, w_proj, b_proj, mode=None, trace=False):
    nc = _get_nc()
    xt, wq_t, wk_t, wv_t, wp_t, bias_rep = _prep_host(x, w_qkv, w_proj, b_proj)
    in_maps = [
        {"xt": np.ascontiguousarray(xt[b]), "wq": wq_t, "wk": wk_t,
         "wv": wv_t, "wp": wp_t, "bias": bias_rep}
        for b in range(B)
    ]
    res = run_bass_kernel_spmd(
        nc, in_maps, core_ids=list(range(B)), trace=trace
    )
    out = np.stack([res.results[b]["y"] for b in range(B)]).astype(np.float32)
    return out, res


def kernel(x, w_qkv, w_proj, b_proj):
    out, _ = run(x, w_qkv, w_proj, b_proj)
    return out
